# revision 29
# baseline (speedup 1.0000x reference)
"""Tensor-parallel LlamaAttention (S=2048, HID=4096, NH=32, NKV=8) on 8 trn2 cores.

Sharding: core c owns q heads {c, c+8, c+16, c+24} (all share kv head c) and
kv head c.  Projections + attention fully local; avT (bf16 [128,2048] per
head) AllGathered per head-group; each core computes its 512 o_proj columns.

v2: single contiguous DMA per weight tensor (host pre-tiled layouts), v-proj
via wv-stationary matmuls + DMA transpose, 2-ktile-batched exp, approx
reciprocal for softmax norm, phase-3 accumulation ordered by head-group so
the last AllGather is hidden behind matmuls of already-arrived groups.
"""

import numpy as np
import ml_dtypes

import concourse.bacc as bacc
import concourse.tile as tile
import concourse.mybir as mybir
from concourse import bass_isa
from concourse.bass_utils import run_bass_kernel_spmd

S = 2048
HID = 4096
NH = 32
NKV = 8
HD = 128
HALF = 64
N_CORES = 8
NREP = NH // NKV  # 4 q heads per core
NHT = HID // 128  # 32 hidden tiles
NST = S // 128    # 16 seq tiles
NSC = S // 512    # 4 seq chunks
BF16 = mybir.dt.bfloat16
F32 = mybir.dt.float32

_CACHE = {}


def build_nc():
    nc = bacc.Bacc("TRN2", target_bir_lowering=False, debug=False,
                   num_devices=N_CORES)

    xt = nc.dram_tensor("xt", [128, NSC * NHT * 512], BF16,
                        kind="ExternalInput").ap()
    wq = nc.dram_tensor("wqT", [128, NREP * NHT * 128], BF16,
                        kind="ExternalInput").ap()
    wk = nc.dram_tensor("wkT", [128, NHT * 128], BF16, kind="ExternalInput").ap()
    wv = nc.dram_tensor("wvT", [128, NHT * 128], BF16, kind="ExternalInput").ap()
    wo = nc.dram_tensor("woT", [128, NHT * 512], BF16, kind="ExternalInput").ap()
    cosT = nc.dram_tensor("cosT", [128, S], BF16, kind="ExternalInput").ap()
    sinN = nc.dram_tensor("sinN", [128, S], BF16, kind="ExternalInput").ap()
    tri = nc.dram_tensor("triT", [128, 128], BF16, kind="ExternalInput").ap()
    ones_c = nc.dram_tensor("ones_c", [128, 128], BF16,
                            kind="ExternalInput").ap()
    ones_r = nc.dram_tensor("ones_r", [1, 128], F32, kind="ExternalInput").ap()

    o_out = nc.dram_tensor("o_out", [128, NST * 512], F32,
                           kind="ExternalOutput").ap()

    # per-head AllGather split into two half-sequence collectives so the
    # first halves launch while phase 2 is still running
    ag_in = [[nc.dram_tensor(f"ag_in{j}_{h}", [HD, S // 2], BF16).ap()
              for h in range(2)] for j in range(NREP)]
    ag_out = [[nc.dram_tensor(f"ag_out{j}_{h}", [N_CORES * HD, S // 2], BF16,
                              addr_space="Shared").ap()
               for h in range(2)] for j in range(NREP)]

    with tile.TileContext(nc) as tc:
        _body(nc, tc, xt, wq, wk, wv, wo, cosT, sinN, tri, ones_c, ones_r,
              o_out, ag_in, ag_out)
    nc.compile()
    return nc


def _body(nc, tc, xt, wq, wk, wv, wo, cosT, sinN, tri, ones_c, ones_r,
          o_out, ag_in, ag_out):
    with (
        tc.tile_pool(name="consts", bufs=1) as cpool,
        tc.tile_pool(name="qkv", bufs=1) as qkvpool,
        tc.tile_pool(name="av", bufs=1) as avpool,
    ):
        tri_sb = cpool.tile([128, 128], BF16, tag="tri")
        onc_sb = cpool.tile([128, 128], BF16, tag="onc")
        onr_sb = cpool.tile([1, 128], F32, tag="onr")
        nc.gpsimd.dma_start(out=tri_sb[:], in_=tri[:])
        nc.gpsimd.dma_start(out=onc_sb[:], in_=ones_c[:])
        nc.gpsimd.dma_start(out=onr_sb[:], in_=ones_r[:])

        qT_sb = [qkvpool.tile([HD, S], BF16, tag=f"qT{j}", name=f"qT{j}")
                 for j in range(NREP)]
        kT_sb = qkvpool.tile([HD, S], BF16, tag="kT")
        v_sb = qkvpool.tile([128, S], BF16, tag="v")  # [seq-in-tile, hd] per kt
        avT_sb = [avpool.tile([HD, S], BF16, tag=f"av{j}", name=f"avT{j}")
                  for j in range(NREP)]

        with (
            tc.tile_pool(name="rconsts", bufs=1) as rcpool,
            tc.tile_pool(name="wproj", bufs=1) as wpool,
            tc.tile_pool(name="xc", bufs=8) as xpool,
            tc.tile_pool(name="rope", bufs=2) as rpool,
            tc.tile_pool(name="vt", bufs=2) as vtpool,
            tc.tile_pool(name="pproj", bufs=4, space="PSUM") as pproj,
        ):
            _phase1(nc, tc, xt, wq, wk, wv, cosT, sinN, qT_sb, kT_sb, v_sb,
                    rcpool, wpool, xpool, rpool, vtpool, pproj)

        with (
            tc.tile_pool(name="wo", bufs=1) as wopool,
            tc.tile_pool(name="agt", bufs=1) as agtpool,
        ):
            wo_sb = wopool.tile([128, NHT * 512], BF16, tag="wo")
            nc.sync.dma_start(out=wo_sb[:], in_=wo[:])

            with (
                tc.tile_pool(name="probs", bufs=8) as ptpool,
                tc.tile_pool(name="small", bufs=2) as spool,
                tc.tile_pool(name="psc", bufs=2, space="PSUM") as psc,
                tc.tile_pool(name="psav", bufs=2, space="PSUM") as psav,
                tc.tile_pool(name="psrs", bufs=2, space="PSUM") as psrs,
            ):
                agt = _phase2(nc, tc, qT_sb, kT_sb, v_sb, avT_sb, tri_sb,
                              onc_sb, onr_sb, ag_in, ag_out, agtpool, ptpool,
                              spool, psc, psav, psrs)

            with (
                tc.tile_pool(name="po", bufs=8, space="PSUM") as po,
                tc.tile_pool(name="osb", bufs=1) as osbpool,
            ):
                _phase3(nc, tc, wo_sb, o_out, agt, po, osbpool)


def _phase1(nc, tc, xt, wq, wk, wv, cosT, sinN, qT_sb, kT_sb, v_sb,
            rcpool, wpool, xpool, rpool, vtpool, pproj):
    cos_sb = rcpool.tile([128, S], BF16, tag="cos")
    sin_sb = rcpool.tile([128, S], BF16, tag="sin")
    wq_sb = wpool.tile([128, NREP * NHT * 128], BF16, tag="wq")
    wk_sb = wpool.tile([128, NHT * 128], BF16, tag="wk")
    wv_sb = wpool.tile([128, NHT * 128], BF16, tag="wv")
    xq0 = [xpool.tile([128, 8 * 512], BF16, tag="xc", name=f"x0q{i}")
           for i in range(4)]

    # DMA completions are tracked by a shared counting semaphore, so a
    # consumer of DMA #k waits for ALL DMAs issued before it.  Order by
    # first-use: wk + chunk-0 x gate the very first matmuls.
    nc.sync.dma_start(out=wk_sb[:], in_=wk[:])
    for i in range(4):
        base = i * 8 * 512
        nc.sync.dma_start(out=xq0[i][:], in_=xt[:, base:base + 8 * 512])
    nc.sync.dma_start(out=wv_sb[:], in_=wv[:])
    nc.sync.dma_start(out=wq_sb[:, 0:NHT * 128], in_=wq[:, 0:NHT * 128])
    nc.sync.dma_start(out=cos_sb[:], in_=cosT[:])
    nc.sync.dma_start(out=sin_sb[:], in_=sinN[:])
    for j in range(1, NREP):
        nc.sync.dma_start(out=wq_sb[:, j * NHT * 128:(j + 1) * NHT * 128],
                          in_=wq[:, j * NHT * 128:(j + 1) * NHT * 128])

    def _rope(dst, pp, sc):
        t_c = rpool.tile([128, 512], F32, tag="tc")
        t_s = rpool.tile([128, 512], F32, tag="ts")
        nc.vector.tensor_mul(t_c[:], pp[:], cos_sb[:, sc])
        nc.vector.tensor_mul(t_s[0:HALF, :], pp[HALF:128, :],
                             sin_sb[0:HALF, sc])
        nc.vector.tensor_mul(t_s[HALF:128, :], pp[0:HALF, :],
                             sin_sb[HALF:128, sc])
        nc.vector.tensor_add(dst[:, sc], t_c[:], t_s[:])

    for cs in range(NSC):
        sc = slice(cs * 512, (cs + 1) * 512)
        if cs == 0:
            xq = xq0
        else:
            xq = [xpool.tile([128, 8 * 512], BF16, tag="xc", name=f"x{cs}q{i}")
                  for i in range(4)]
            for i in range(4):
                base = (cs * NHT + i * 8) * 512
                nc.sync.dma_start(out=xq[i][:], in_=xt[:, base:base + 8 * 512])

        def xs(h):
            return xq[h // 8][:, (h % 8) * 512:(h % 8 + 1) * 512]

        pk = pproj.tile([128, 512], F32, tag="mm", name=f"pk{cs}")
        for h in range(NHT):
            nc.tensor.matmul(pk[:], wk_sb[:, h * 128:(h + 1) * 128], xs(h),
                             start=(h == 0), stop=(h == NHT - 1))
        _rope(kT_sb, pk, sc)

        pv = pproj.tile([128, 512], F32, tag="mm", name=f"pv{cs}")
        for h in range(NHT):
            nc.tensor.matmul(pv[:], wv_sb[:, h * 128:(h + 1) * 128], xs(h),
                             start=(h == 0), stop=(h == NHT - 1))
        vt = vtpool.tile([128, 512], BF16, tag="vt", name=f"vt{cs}")
        nc.vector.tensor_copy(vt[:], pv[:])
        # scalar queue: sync is busy with weight/x DMAs whose WAR deps would
        # serialize these transposes behind end-of-phase-1
        for t in range(4):
            nc.scalar.dma_start_transpose(
                v_sb[:, cs * 512 + t * 128: cs * 512 + (t + 1) * 128],
                vt[:, t * 128:(t + 1) * 128])

        for j in range(NREP):
            pq = pproj.tile([128, 512], F32, tag="mm", name=f"pq{cs}_{j}")
            for h in range(NHT):
                nc.tensor.matmul(
                    pq[:], wq_sb[:, (j * NHT + h) * 128:(j * NHT + h + 1) * 128],
                    xs(h), start=(h == 0), stop=(h == NHT - 1))
            _rope(qT_sb[j], pq, sc)


def _phase2(nc, tc, qT_sb, kT_sb, v_sb, avT_sb, tri_sb, onc_sb, onr_sb,
            ag_in, ag_out, agtpool, ptpool, spool, psc, psav, psrs):
    """Attention with two heads' pipelines interleaved: while one head's pair
    is in the ACT exp stage, the other head's matmuls keep the PE busy, so
    the PE<->ACT ping-pong of a single dependency chain disappears."""
    Exp = mybir.ActivationFunctionType.Exp

    def emit_pair(j, C, kp, nkt, pav, prs):
        qc0 = C * 512
        a, b = 2 * kp, 2 * kp + 1
        offa = max(0, (a - 4 * C) * 128)
        offb = max(0, (b - 4 * C) * 128)
        ps2 = psc.tile([128, 1024], F32, tag="sc", name=f"ps{j}_{C}_{kp}")
        nc.tensor.matmul(ps2[:, offa:512],
                         kT_sb[:, a * 128:(a + 1) * 128],
                         qT_sb[j][:, qc0 + offa:qc0 + 512],
                         start=True, stop=True)
        nc.tensor.matmul(ps2[:, 512 + offb:1024],
                         kT_sb[:, b * 128:(b + 1) * 128],
                         qT_sb[j][:, qc0 + offb:qc0 + 512],
                         start=True, stop=True)
        pt2 = ptpool.tile([128, 1024], BF16, tag="pt", name=f"pt{j}_{C}_{kp}")
        if offa == 0 and offb == 0:
            nc.scalar.activation(pt2[:], ps2[:], Exp)
        else:
            nc.scalar.activation(pt2[:, offa:1024], ps2[:, offa:1024], Exp)
            if offb > offa:
                nc.vector.memset(pt2[:, 512 + offa:512 + offb], 0.0)
        if a >= 4 * C:
            nc.vector.tensor_mul(pt2[:, offa:offa + 128],
                                 pt2[:, offa:offa + 128], tri_sb[:])
        if b >= 4 * C:
            nc.vector.tensor_mul(pt2[:, 512 + offb:512 + offb + 128],
                                 pt2[:, 512 + offb:512 + offb + 128],
                                 tri_sb[:])
        nc.tensor.matmul(pav[:, offa:512], v_sb[:, a * 128:(a + 1) * 128],
                         pt2[:, offa:512], start=(a == 0), stop=False)
        nc.tensor.matmul(pav[:, offb:512], v_sb[:, b * 128:(b + 1) * 128],
                         pt2[:, 512 + offb:1024],
                         start=False, stop=(b == nkt - 1))
        rsp = ptpool.tile([128, 512], BF16, tag="rsp", name=f"rsp{j}_{C}_{kp}")
        nc.vector.tensor_add(rsp[:, offa:512], pt2[:, offa:512],
                             pt2[:, 512 + offa:1024])
        nc.tensor.matmul(prs[:, offa:512], onc_sb[:], rsp[:, offa:512],
                         start=(kp == 0), stop=(kp == nkt // 2 - 1))

    def emit_norm(j, C, pav, prs):
        qc0 = C * 512
        bsb = spool.tile([128, 512], F32, tag="bsb", name=f"bs{j}_{C}")
        nc.vector.reciprocal_approx_fast(out=bsb[:], in_=prs[:])
        nc.vector.tensor_mul(avT_sb[j][:, qc0:qc0 + 512], pav[:], bsb[:])

    agt = []
    for g in range(NREP // 2):
        ja, jb = 2 * g, 2 * g + 1
        for C in range(NSC):
            nkt = 4 * C + 4
            pav_a = psav.tile([128, 512], F32, tag="av", name=f"pav{ja}_{C}")
            pav_b = psav.tile([128, 512], F32, tag="av", name=f"pav{jb}_{C}")
            prs_a = psrs.tile([128, 512], F32, tag="rs", name=f"prs{ja}_{C}")
            prs_b = psrs.tile([128, 512], F32, tag="rs", name=f"prs{jb}_{C}")
            # stagger stream b by one pair: each stream's chunk-end norm
            # overlaps the other stream's matmul work
            for kp in range(nkt // 2):
                emit_pair(ja, C, kp, nkt, pav_a, prs_a)
                if kp >= 1:
                    emit_pair(jb, C, kp - 1, nkt, pav_b, prs_b)
            emit_norm(ja, C, pav_a, prs_a)
            emit_pair(jb, C, nkt // 2 - 1, nkt, pav_b, prs_b)
            emit_norm(jb, C, pav_b, prs_b)
            if C in (1, 3):
                h = C // 2
                hs = slice(h * (S // 2), (h + 1) * (S // 2))
                for j in (ja, jb):
                    nc.gpsimd.dma_start(out=ag_in[j][h][:],
                                        in_=avT_sb[j][:, hs])
                    nc.gpsimd.collective_compute(
                        "AllGather", mybir.AluOpType.bypass,
                        replica_groups=[list(range(N_CORES))],
                        ins=[ag_in[j][h][:]], outs=[ag_out[j][h][:]])
        # 24 physical slots shared by tag: group 3 reuses group 0's slots,
        # so its DMA waits until phase-3 finishes reading group 0.
        for j in (ja, jb):
            agt_j = [agtpool.tile([128, S], BF16, tag=f"ag{(j % 3) * 8 + r}",
                                  name=f"ag{j}_{r}") for r in range(N_CORES)]
            for r in range(N_CORES):
                for h in range(2):
                    nc.sync.dma_start(
                        out=agt_j[r][:, h * (S // 2):(h + 1) * (S // 2)],
                        in_=ag_out[j][h][r * 128:(r + 1) * 128, :])
            agt.append(agt_j)
    return agt


def _phase3(nc, tc, wo_sb, o_out, agt, po, osbpool):
    osb = osbpool.tile([128, NST * 512], F32, tag="osb")
    for j in range(NREP):
        for st in range(NST):
            pj = po.tile([128, 512], F32, tag="po", name=f"po{j}_{st}")
            for r in range(N_CORES):
                i = j * N_CORES + r
                nc.tensor.matmul(pj[:],
                                 agt[j][r][:, st * 128:(st + 1) * 128],
                                 wo_sb[:, i * 512:(i + 1) * 512],
                                 start=(r == 0), stop=(r == N_CORES - 1))
            sl = slice(st * 512, (st + 1) * 512)
            if j == 0:
                nc.vector.tensor_copy(osb[:, sl], pj[:])
            else:
                nc.vector.tensor_add(osb[:, sl], osb[:, sl], pj[:])
            if j == NREP - 1:
                # stream the finished column block out right away (scalar
                # queue: sync may be blocked behind group-3 agt DMAs)
                nc.scalar.dma_start(out=o_out[:, sl], in_=osb[:, sl])


def prep_inputs(hidden_states, wq, wk, wv, wo, cos, sin, causal_mask=None):
    bf16 = ml_dtypes.bfloat16
    x = np.asarray(hidden_states, np.float32)[0]          # (S, HID)
    xT = np.ascontiguousarray(x.T).astype(bf16)           # (HID, S)
    # tiled x: [p, (cs*32+h)*512 + c] = xT[h*128+p, cs*512+c]
    xt = np.ascontiguousarray(
        xT.reshape(NHT, 128, NSC, 512).transpose(1, 2, 0, 3).reshape(
            128, NSC * NHT * 512))
    wq_s = (np.asarray(wq, np.float32) / np.sqrt(HD)).astype(np.float32)
    cos2 = np.asarray(cos, np.float32)[0, 0]              # (S, 64)
    sin2 = np.asarray(sin, np.float32)[0, 0]
    cosT = np.concatenate([cos2.T, cos2.T], 0).astype(bf16)   # (128, S)
    sinN = np.concatenate([-sin2.T, sin2.T], 0).astype(bf16)  # (128, S)
    kl = np.arange(128)[:, None]
    ql = np.arange(128)[None, :]
    triT = (kl <= ql).astype(bf16)                        # allow k <= q
    ones_c = np.ones((128, 128), bf16)
    ones_r = np.ones((1, 128), np.float32)

    # wo reordered to AllGather row order: row p = j*1024 + r*128 + d
    j_ = np.arange(NREP)[:, None, None]
    r_ = np.arange(N_CORES)[None, :, None]
    d_ = np.arange(HD)[None, None, :]
    col_order = ((j_ * N_CORES + r_) * HD + d_).reshape(-1)
    woT_full = np.ascontiguousarray(
        np.asarray(wo, np.float32)[:, col_order].T).astype(bf16)  # (4096, 4096)

    def tile128(a, blk):
        # (4096, blk) -> (128, 32*blk), block h at col h*blk
        return np.ascontiguousarray(
            a.reshape(NHT, 128, blk).transpose(1, 0, 2).reshape(128, NHT * blk))

    in_maps = []
    for c in range(N_CORES):
        heads = [jj * N_CORES + c for jj in range(NREP)]
        wq_rows = np.concatenate([wq_s[h * HD:(h + 1) * HD, :] for h in heads], 0)
        wqT_c = np.ascontiguousarray(wq_rows.T).astype(bf16)        # (HID, 512)
        # per-j blocks of 32*128 cols: [p, (j*32+h)*128 + d]
        wq_t = np.ascontiguousarray(
            wqT_c.reshape(NHT, 128, NREP, 128).transpose(1, 2, 0, 3).reshape(
                128, NREP * NHT * 128))
        wkT_c = np.ascontiguousarray(
            np.asarray(wk, np.float32)[c * HD:(c + 1) * HD, :].T).astype(bf16)
        wvT_c = np.ascontiguousarray(
            np.asarray(wv, np.float32)[c * HD:(c + 1) * HD, :].T).astype(bf16)
        wk_t = tile128(wkT_c, 128)
        wv_t = tile128(wvT_c, 128)
        wo_t = tile128(np.ascontiguousarray(woT_full[:, c * 512:(c + 1) * 512]),
                       512)
        in_maps.append(dict(xt=xt, wqT=wq_t, wkT=wk_t, wvT=wv_t, woT=wo_t,
                            cosT=cosT, sinN=sinN, triT=triT,
                            ones_c=ones_c, ones_r=ones_r))
    return in_maps


def postprocess(results):
    out = np.empty((S, HID), np.float32)
    for c in range(N_CORES):
        o = results[c]["o_out"].reshape(128, NST, 512).transpose(1, 0, 2)
        out[:, c * 512:(c + 1) * 512] = o.reshape(S, 512)
    return out[None]


def get_nc():
    if "nc" not in _CACHE:
        _CACHE["nc"] = build_nc()
    return _CACHE["nc"]


def kernel(hidden_states, wq, wk, wv, wo, cos, sin, causal_mask=None):
    nc = get_nc()
    in_maps = prep_inputs(hidden_states, wq, wk, wv, wo, cos, sin, causal_mask)
    res = run_bass_kernel_spmd(nc, in_maps, core_ids=list(range(N_CORES)))
    return postprocess(res.results)
